# revision 16
# baseline (speedup 1.0000x reference)
"""Capsule-routing kernel (einsum bni,nkdi,nk->bkd + squash) on 8 trn2 cores.

Sharding: over the contraction axis n (2048 -> 256 per core).  Each core
reads only its slice of x and W -- every input byte is read exactly once
machine-wide.  Each core emits a partial s[b,(k,d)] over its n-slice; the
host sums the 8 partials and applies the tiny squash nonlinearity.

v2 changes vs the 40.2us baseline (trace-driven):
  - W is shipped as int8 with a per-(n,k) host-computed scale a_w folded
    into the softmax(R) multiplier: wb = int8(W) * bf16(Rs*a_w).  Halves
    the dominant HBM stream (4MB -> 2MB/core); measured end-to-end rel
    err ~0.8% (gate 2e-2).
  - ALL input DMAs ride ONE HWDGE queue (sync) in explicit arrival
    order: rs, x0a, W0c0, x0b, W0c1, W0c2, x1, W1c0..2.  The old 2-queue
    layout round-robined x behind W, landing x0 at t=18.5us and stalling
    the first matmul.  Single-queue FIFO lands x0a ~4us in.
  - Rs is uploaded un-broadcast [n, k] (16KB not 256KB); the scale op
    broadcasts over (i, d) with stride-0 AP dims.
  - PE warm-up: ~36 dummy 128x128 matmuls on the identity tile right
    after the preamble keep the PE HAM busy so it unthrottles to 2.4GHz
    (K=8/8) before the first real matmul (trace showed the first ~14
    real matmuls ran at 1.2GHz).
  - Output DMAs go SWDGE (gpsimd, per-DMA semaphores) because the 10
    input DMAs exhaust the 8 HWDGE DMAHW lanes and a reused lane would
    add a second sem-wait (illegal in this walrus build).
  - acc0 (B rows 0..127) finishes 3 matmuls early (tail reordered
    h0,h0,h1,h1) so its PSUM evac + output DMA overlap acc1's tail.
  - Tile's kernel-sem range narrowed (teardown probe): the NEFF epilogue
    clears every semaphore one instruction at a time (~6.6us!); if the
    clear range follows the declared range this shrinks it.

The walrus build in this container accepts at most ONE sync-wait per
instruction.  Consequences handled here:
  - tiny DVE "toucher" ops absorb the rs/x DMA completions into DVE
    program order (so matmuls and scale ops carry at most one wait)
  - W-chunk scale ops carry their chunk's DMA wait directly (their other
    operand rs is already DVE-ordered)
  - input DMAs may reuse DMAHW lanes (their only wait); output DMAs are
    SWDGE so their single wait is the evac dependency
  - Tile's multi-wait kernel-tail drain is monkeypatched into a chain of
    single-wait drains
"""

import os
import sys

import numpy as np

if "/opt/trn_rl_repo" not in sys.path:
    sys.path.insert(0, "/opt/trn_rl_repo")

import bass_rust as _bass_rust
import concourse.bass as bass
import concourse.mybir as mybir
import ml_dtypes
from concourse.bass_utils import run_bass_kernel_spmd
from concourse.masks import make_identity
from concourse.tile import TileContext

NCORES = 8
B, N, I = 256, 2048, 16
K, D = 32, 16
NL = N // NCORES  # 256 n-values per core
KD = K * D  # 512
F_W = I * K * D  # 8192   (i-major W layout)
F_X = I * B  # 4096      (x^T layout: [n, i, B])
EPS = 1e-7

FP32 = mybir.dt.float32
BF16 = mybir.dt.bfloat16
INT8 = mybir.dt.int8
NPBF16 = ml_dtypes.bfloat16

N_WARM_MM = 40  # dummy matmuls to unthrottle the PE HAM before real work

# W chunk boundaries in units of i, per 128-partition tile.  Tile 1's
# last chunk is a single i-slice so the final DMA-sem wait + scale +
# matmul tail past the last HBM byte is as short as possible.
WCHUNKS0 = [(0, 6), (6, 11), (11, 16)]
WCHUNKS1 = [(0, 5), (5, 10), (10, 15), (15, 16)]

# Split Tile's multi-wait kernel-tail drain into a chain of single-wait
# drains (program order on the sync sequencer makes the chain equivalent).
if not getattr(TileContext, "_split_drain_patched", False):

    def _split_drain_and_barrier(self, tick_clock, wait_clock):
        gc = tick_clock.global_clock
        vals = list(gc)
        for j, v in enumerate(vals):
            if v > 0:
                sub = [0] * len(vals)
                sub[j] = v
                d = self.nc.sync.drain()
                wait_clock.add_sem_waits(
                    d.ins,
                    _bass_rust.ScopedClock({None: _bass_rust.VectorClock(sub)}),
                )
        self.nc.all_engine_barrier()
        assert self.sems is not None
        popped = self.nc._tile_sem_poison_stack.pop()
        assert popped is self._sem_poison
        self.nc.clear_and_free_semaphores(list(self.sems.allocated().values()))

    TileContext._drain_and_barrier = _split_drain_and_barrier
    TileContext._split_drain_patched = True


def build_bass() -> bass.Bass:
    nc = bass.Bass()
    x_d = nc.dram_tensor("xs", [NL, F_X], BF16, kind="ExternalInput")
    w_d = nc.dram_tensor("ws", [NL, F_W], BF16, kind="ExternalInput")
    r_d = nc.dram_tensor("rs", [NL, K], BF16, kind="ExternalInput")
    o_d = nc.dram_tensor("out", [B, KD], BF16, kind="ExternalOutput")

    with TileContext(nc) as tc:
        with (
            tc.tile_pool(name="big", bufs=1) as big,
            tc.tile_pool(name="ps_warm", bufs=1, space="PSUM") as ps_warm,
            tc.tile_pool(name="ps_acc", bufs=1, space="PSUM") as ps_acc,
        ):
            rs_kd = big.tile([128, 2 * K], BF16, tag="rs_kd")
            xb = [big.tile([128, F_X], BF16, tag=f"x{t}", name=f"x{t}") for t in range(2)]
            wc = [big.tile([128, F_W], BF16, tag=f"w{t}", name=f"w{t}") for t in range(2)]
            wb = [big.tile([128, F_W], BF16, tag=f"wb{t}", name=f"wb{t}") for t in range(2)]

            # ---- input DMAs.  rs + x-tile-0 ride the sync HWDGE queue;
            # W and x-tile-1 ride the gpsimd SWDGE queue (per-DMA
            # semaphores), FIFO-ordered W0, x1, W1 so tile-1 data lands
            # mid-stream, not last (the queues round-robin ~50/50). ----
            nc.sync.dma_start(
                out=rs_kd[:], in_=r_d.rearrange("(t p) k -> p t k", t=2)
            )
            nc.sync.dma_start(
                out=xb[0][:, : 8 * B], in_=x_d[0:128, : 8 * B]
            )
            nc.sync.dma_start(
                out=xb[0][:, 8 * B :], in_=x_d[0:128, 8 * B :]
            )

            # ---- PE warm-up: identity (gpsimd) -> transpose absorbs the
            # gpsimd dep into PE order -> dummy matmuls keep HAM busy ----
            identb = big.tile([128, 128], BF16, tag="identb")
            make_identity(nc, identb)

            def dma_w(t, chunk):
                i0, i1 = chunk
                nc.gpsimd.dma_start(
                    out=wc[t][:, i0 * KD : i1 * KD],
                    in_=w_d[t * 128 : (t + 1) * 128, i0 * KD : i1 * KD],
                )

            for chunk in WCHUNKS0:
                dma_w(0, chunk)
            nc.gpsimd.dma_start(out=xb[1][:], in_=x_d[128:256, :])
            for chunk in WCHUNKS1:
                dma_w(1, chunk)
            warm_tp = ps_warm.tile([128, 128], BF16, tag="warmtp")
            nc.tensor.transpose(warm_tp[:], identb[:], identb[:])
            warm_mm = ps_warm.tile([128, 128], FP32, tag="warmmm")
            for _ in range(N_WARM_MM):
                nc.tensor.matmul(
                    warm_mm[:], identb[:], identb[:], start=True, stop=True
                )

            # ---- DVE pipeline in arrival order: touchers (absorb rs/x
            # DMA completions) interleaved with per-chunk scale ops ----
            def touch(name, src):
                tt = big.tile([128, 1], BF16, tag=f"touch_{name}")
                nc.vector.tensor_copy(tt[:], src)

            def scale(t, chunk):
                i0, i1 = chunk
                ni = i1 - i0
                sl_in = wc[t][:, i0 * KD : i1 * KD].rearrange(
                    "p (i k d) -> p i k d", k=K, d=D
                )
                sl_out = wb[t][:, i0 * KD : i1 * KD].rearrange(
                    "p (i k d) -> p i k d", k=K, d=D
                )
                r_sl = rs_kd[:, t * K : (t + 1) * K]
                r_b = bass.AP(
                    tensor=r_sl.tensor,
                    offset=r_sl.offset,
                    ap=[r_sl.ap[0], [0, ni], [1, K], [0, D]],
                )
                nc.vector.tensor_mul(sl_out, sl_in, r_b)

            touch("rs", rs_kd[:, 0:1])
            touch("x0a", xb[0][:, 0:1])
            scale(0, WCHUNKS0[0])
            touch("x0b", xb[0][:, 8 * B : 8 * B + 1])
            scale(0, WCHUNKS0[1])
            scale(0, WCHUNKS0[2])
            touch("x1", xb[1][:, 0:1])
            for chunk in WCHUNKS1:
                scale(1, chunk)

            # ---- main matmuls ----
            # acc_h[b, (k d)] += xb[t][:, (i, h-half)]^T @ wb[t][:, i-slice].
            # Tail reordered h0,h0,h1,h1 so acc0 finalizes early and its
            # evac + output DMA overlap acc1's last matmuls.
            accs = [
                ps_acc.tile([128, KD], FP32, tag=f"acc{h}", name=f"acc{h}")
                for h in range(2)
            ]

            def mm(t, i, h, start, stop):
                rhs = wb[t][:, i * KD : (i + 1) * KD]
                lhsT = xb[t][:, i * B + h * 128 : i * B + (h + 1) * 128]
                nc.tensor.matmul(accs[h][:], lhsT, rhs, start=start, stop=stop)

            for t in range(2):
                for i in range(I):
                    if t == 1 and i == I - 1:
                        continue
                    first = t == 0 and i == 0
                    mm(t, i, 0, first, False)
                    mm(t, i, 1, first, False)
            mm(1, I - 1, 0, False, True)
            mm(1, I - 1, 1, False, True)

            # ---- output: PSUM -> SBUF bf16 on DVE, one HWDGE out DMA on
            # a fresh DMAHW lane (sync carried only 3 input DMAs) ----
            o_sb = big.tile([128, 2 * KD], BF16, tag="osb")
            for h in range(2):
                nc.vector.tensor_copy(o_sb[:, h * KD : (h + 1) * KD], accs[h][:])
            nc.sync.dma_start(
                out=o_d.rearrange("(h p) f -> p h f", h=2), in_=o_sb[:]
            )

    return nc


_CACHE: dict = {}

# test.py sets these for profiling; harness never touches them.
LAST_RESULTS = None


def _trace_kwargs():
    if os.environ.get("BASS_KERNEL_TRACE") == "1":
        cores = os.environ.get("BASS_KERNEL_TRACE_CORES", "0")
        return dict(trace=True, trace_cores=[int(c) for c in cores.split(",")])
    return {}


def kernel(x: np.ndarray, W: np.ndarray, R: np.ndarray) -> np.ndarray:
    global LAST_RESULTS
    x = np.asarray(x, dtype=np.float32)
    W = np.asarray(W, dtype=np.float32)
    R = np.asarray(R, dtype=np.float32)

    # softmax over n (65K elements -- host)
    Rm = R.max(axis=0, keepdims=True)
    e = np.exp(R - Rm)
    Rs = (e / e.sum(axis=0, keepdims=True)).astype(np.float32)

    # upload layouts: x^T as [n, i, B], W i-major as [n, i, k, d], Rs
    # un-broadcast [n, k]; all in the kernel's bf16 compute precision
    Wp = (
        np.ascontiguousarray(W.transpose(0, 3, 1, 2)).reshape(N, F_W).astype(NPBF16)
    )
    Rp = np.ascontiguousarray(Rs).astype(NPBF16)  # [N, K]
    Xp = np.ascontiguousarray(x.transpose(1, 2, 0)).reshape(N, F_X).astype(NPBF16)

    in_maps = []
    for c in range(NCORES):
        sl = slice(c * NL, (c + 1) * NL)
        in_maps.append({"xs": Xp[sl], "ws": Wp[sl], "rs": Rp[sl]})

    if "nc" not in _CACHE:
        _CACHE["nc"] = build_bass()
    nc = _CACHE["nc"]

    res = run_bass_kernel_spmd(
        nc, in_maps, core_ids=list(range(NCORES)), **_trace_kwargs()
    )
    LAST_RESULTS = res

    s = np.zeros((B, KD), np.float32)
    for r in res.results:
        s += r["out"].astype(np.float32)
    s = s.reshape(B, K, D)
    sq = np.sum(np.square(s), axis=-1, keepdims=True) + EPS
    v = (np.sqrt(sq) / (1.0 + sq)) * s
    return v.astype(np.float32)


if __name__ == "__main__":
    rng = np.random.default_rng(0)
    x = rng.standard_normal((B, N, I), dtype=np.float32)
    W = (rng.standard_normal((N, K, D, I), dtype=np.float32) * 0.05).astype(np.float32)
    R = rng.standard_normal((N, K), dtype=np.float32)
    out = kernel(x, W, R)
    print("out", out.shape, out.dtype, float(np.abs(out).mean()))


# revision 21
# speedup vs baseline: 1.0748x; 1.0748x over previous
"""Capsule-routing kernel (einsum bni,nkdi,nk->bkd + squash) on 8 trn2 cores.

Sharding: over the contraction axis n (2048 -> 256 per core).  Each core
reads only its slice of x and W -- every input byte is read exactly once
machine-wide.  Each core emits a partial s[b,(k,d)] over its n-slice; the
host sums the 8 partials and applies the tiny squash nonlinearity.

v2 changes vs the 40.2us baseline (trace-driven):
  - W is shipped as int8 with a per-(n,k) host-computed scale a_w folded
    into the softmax(R) multiplier: wb = int8(W) * bf16(Rs*a_w).  Halves
    the dominant HBM stream (4MB -> 2MB/core); measured end-to-end rel
    err ~0.8% (gate 2e-2).
  - ALL input DMAs ride ONE HWDGE queue (sync) in explicit arrival
    order: rs, x0a, W0c0, x0b, W0c1, W0c2, x1, W1c0..2.  The old 2-queue
    layout round-robined x behind W, landing x0 at t=18.5us and stalling
    the first matmul.  Single-queue FIFO lands x0a ~4us in.
  - Rs is uploaded un-broadcast [n, k] (16KB not 256KB); the scale op
    broadcasts over (i, d) with stride-0 AP dims.
  - PE warm-up: ~36 dummy 128x128 matmuls on the identity tile right
    after the preamble keep the PE HAM busy so it unthrottles to 2.4GHz
    (K=8/8) before the first real matmul (trace showed the first ~14
    real matmuls ran at 1.2GHz).
  - Output DMAs go SWDGE (gpsimd, per-DMA semaphores) because the 10
    input DMAs exhaust the 8 HWDGE DMAHW lanes and a reused lane would
    add a second sem-wait (illegal in this walrus build).
  - acc0 (B rows 0..127) finishes 3 matmuls early (tail reordered
    h0,h0,h1,h1) so its PSUM evac + output DMA overlap acc1's tail.
  - Tile's kernel-sem range narrowed (teardown probe): the NEFF epilogue
    clears every semaphore one instruction at a time (~6.6us!); if the
    clear range follows the declared range this shrinks it.

The walrus build in this container accepts at most ONE sync-wait per
instruction.  Consequences handled here:
  - tiny DVE "toucher" ops absorb the rs/x DMA completions into DVE
    program order (so matmuls and scale ops carry at most one wait)
  - W-chunk scale ops carry their chunk's DMA wait directly (their other
    operand rs is already DVE-ordered)
  - input DMAs may reuse DMAHW lanes (their only wait); output DMAs are
    SWDGE so their single wait is the evac dependency
  - Tile's multi-wait kernel-tail drain is monkeypatched into a chain of
    single-wait drains
"""

import os
import sys

import numpy as np

if "/opt/trn_rl_repo" not in sys.path:
    sys.path.insert(0, "/opt/trn_rl_repo")

import bass_rust as _bass_rust
import concourse.bass as bass
import concourse.mybir as mybir
import ml_dtypes
from concourse.bass_utils import run_bass_kernel_spmd
from concourse.masks import make_identity
from concourse.tile import TileContext

NCORES = 8
B, N, I = 256, 2048, 16
K, D = 32, 16
NL = N // NCORES  # 256 n-values per core
KD = K * D  # 512
F_W = I * K * D  # 8192   (i-major W layout)
F_X = I * B  # 4096      (x^T layout: [n, i, B])
EPS = 1e-7

FP32 = mybir.dt.float32
BF16 = mybir.dt.bfloat16
INT8 = mybir.dt.int8
NPBF16 = ml_dtypes.bfloat16

N_WARM_MM = 40  # dummy matmuls to unthrottle the PE HAM before real work

# W chunk boundaries in units of i, per 128-partition tile.  Tile 1's
# chunks shrink toward the end: each chunk pays a fixed ~0.9us DMA-sem
# receipt + scale latency after its last byte, so small tail chunks keep
# the post-stream critical chain short.
WCHUNKS0 = [(0, 4), (4, 8), (8, 12), (12, 16)]
WCHUNKS1 = [(0, 4), (4, 7), (7, 9), (9, 11), (11, 13), (13, 15), (15, 16)]

# Split Tile's multi-wait kernel-tail drain into a chain of single-wait
# drains (program order on the sync sequencer makes the chain equivalent).
if not getattr(TileContext, "_split_drain_patched", False):

    def _split_drain_and_barrier(self, tick_clock, wait_clock):
        gc = tick_clock.global_clock
        vals = list(gc)
        for j, v in enumerate(vals):
            if v > 0:
                sub = [0] * len(vals)
                sub[j] = v
                d = self.nc.sync.drain()
                wait_clock.add_sem_waits(
                    d.ins,
                    _bass_rust.ScopedClock({None: _bass_rust.VectorClock(sub)}),
                )
        self.nc.all_engine_barrier()
        assert self.sems is not None
        popped = self.nc._tile_sem_poison_stack.pop()
        assert popped is self._sem_poison
        self.nc.clear_and_free_semaphores(list(self.sems.allocated().values()))

    TileContext._drain_and_barrier = _split_drain_and_barrier
    TileContext._split_drain_patched = True


def build_bass() -> bass.Bass:
    nc = bass.Bass()
    x_d = nc.dram_tensor("xs", [NL, F_X], BF16, kind="ExternalInput")
    w_d = nc.dram_tensor("ws", [NL, F_W], BF16, kind="ExternalInput")
    r_d = nc.dram_tensor("rs", [NL, K], BF16, kind="ExternalInput")
    o_d = nc.dram_tensor("out", [B, KD], BF16, kind="ExternalOutput")

    with TileContext(nc) as tc:
        with (
            tc.tile_pool(name="big", bufs=1) as big,
            tc.tile_pool(name="ps_warm", bufs=1, space="PSUM") as ps_warm,
            tc.tile_pool(name="ps_acc", bufs=1, space="PSUM") as ps_acc,
        ):
            rs_kd = big.tile([128, 2 * K], BF16, tag="rs_kd")
            rse = big.tile([128, 2 * KD], BF16, tag="rse")
            xb = [big.tile([128, F_X], BF16, tag=f"x{t}", name=f"x{t}") for t in range(2)]
            wc = [big.tile([128, F_W], BF16, tag=f"w{t}", name=f"w{t}") for t in range(2)]
            wb = [big.tile([128, F_W], BF16, tag=f"wb{t}", name=f"wb{t}") for t in range(2)]

            # ---- input DMAs: ALL on the gpsimd SWDGE queue, which the
            # SDMA arbiter drains ahead of the HWDGE queues (measured
            # ~6:1), in exact consumption order.  Per-DMA semaphores, so
            # no DMAHW-lane-reuse waits either. ----
            identb = big.tile([128, 128], BF16, tag="identb")

            def dma_w(t, chunk):
                i0, i1 = chunk
                nc.gpsimd.dma_start(
                    out=wc[t][:, i0 * KD : i1 * KD],
                    in_=w_d[t * 128 : (t + 1) * 128, i0 * KD : i1 * KD],
                )

            nc.gpsimd.dma_start(
                out=rs_kd[:], in_=r_d.rearrange("(t p) k -> p t k", t=2)
            )
            nc.gpsimd.dma_start(
                out=xb[0][:, : 8 * B], in_=x_d[0:128, : 8 * B]
            )
            dma_w(0, WCHUNKS0[0])
            # identity for the PE warm-up burst, squeezed between the
            # head dispatches (so warm-up spans the gap until the first
            # real matmul) and the remaining dispatches
            make_identity(nc, identb)
            nc.gpsimd.dma_start(
                out=xb[0][:, 8 * B :], in_=x_d[0:128, 8 * B :]
            )
            for chunk in WCHUNKS0[1:]:
                dma_w(0, chunk)
            nc.gpsimd.dma_start(out=xb[1][:], in_=x_d[128:256, :])
            for chunk in WCHUNKS1:
                dma_w(1, chunk)
            warm_tp = ps_warm.tile([128, 128], BF16, tag="warmtp")
            nc.tensor.transpose(warm_tp[:], identb[:], identb[:])
            warm_mm = ps_warm.tile([128, 128], FP32, tag="warmmm")
            for _ in range(N_WARM_MM):
                nc.tensor.matmul(
                    warm_mm[:], identb[:], identb[:], start=True, stop=True
                )

            # ---- DVE pipeline in arrival order: touchers (absorb rs/x
            # DMA completions) interleaved with per-chunk scale ops ----
            def touch(name, src):
                tt = big.tile([128, 1], BF16, tag=f"touch_{name}")
                nc.vector.tensor_copy(tt[:], src)

            def scale(t, chunk):
                # toucher first: absorbs the chunk's DMA completion into
                # DVE program order, so the scale op's only sem wait is
                # the DVE counter (covers the rse dep too -- same sem)
                i0, i1 = chunk
                ni = i1 - i0
                touch(f"w{t}_{i0}", wc[t][:, i0 * KD : i0 * KD + 1])
                sl_in = wc[t][:, i0 * KD : i1 * KD].rearrange(
                    "p (i f) -> p i f", f=KD
                )
                sl_out = wb[t][:, i0 * KD : i1 * KD].rearrange(
                    "p (i f) -> p i f", f=KD
                )
                r_sl = rse[:, t * KD : (t + 1) * KD]
                r_b = bass.AP(
                    tensor=r_sl.tensor,
                    offset=r_sl.offset,
                    ap=[r_sl.ap[0], [0, ni], [1, KD]],
                )
                nc.vector.tensor_mul(sl_out, sl_in, r_b)

            touch("rs", rs_kd[:, 0:1])
            # broadcast Rs over d on-device, once (128K elems): rse[p,
            # (t k d)] = rs_kd[p, (t k)]; the per-chunk scale ops then use
            # the dense stride-1 form that keeps DVE in 2x perf mode
            src = rs_kd[:]
            r_src = bass.AP(
                tensor=src.tensor,
                offset=src.offset,
                ap=[src.ap[0], [1, 2 * K], [0, D]],
            )
            nc.vector.tensor_copy(rse[:].rearrange("p (k d) -> p k d", d=D), r_src)
            touch("x0a", xb[0][:, 0:1])
            scale(0, WCHUNKS0[0])
            touch("x0b", xb[0][:, 8 * B : 8 * B + 1])
            scale(0, WCHUNKS0[1])
            scale(0, WCHUNKS0[2])
            scale(0, WCHUNKS0[3])
            touch("x1", xb[1][:, 0:1])
            for chunk in WCHUNKS1:
                scale(1, chunk)

            # ---- main matmuls ----
            # acc_h[b, (k d)] += xb[t][:, (i, h-half)]^T @ wb[t][:, i-slice].
            # Tail reordered h0,h0,h1,h1 so acc0 finalizes early and its
            # evac + output DMA overlap acc1's last matmuls.
            accs = [
                ps_acc.tile([128, KD], FP32, tag=f"acc{h}", name=f"acc{h}")
                for h in range(2)
            ]

            def mm(t, i, h, start, stop):
                rhs = wb[t][:, i * KD : (i + 1) * KD]
                lhsT = xb[t][:, i * B + h * 128 : i * B + (h + 1) * 128]
                nc.tensor.matmul(accs[h][:], lhsT, rhs, start=start, stop=stop)

            for t in range(2):
                for i in range(I):
                    if t == 1 and i == I - 1:
                        continue
                    first = t == 0 and i == 0
                    mm(t, i, 0, first, False)
                    mm(t, i, 1, first, False)
            mm(1, I - 1, 0, False, True)
            mm(1, I - 1, 1, False, True)

            # ---- output: PSUM -> SBUF bf16 on DVE, one HWDGE out DMA on
            # a fresh DMAHW lane (sync carried only 3 input DMAs) ----
            o_sb = big.tile([128, 2 * KD], BF16, tag="osb")
            for h in range(2):
                nc.vector.tensor_copy(o_sb[:, h * KD : (h + 1) * KD], accs[h][:])
            nc.sync.dma_start(
                out=o_d.rearrange("(h p) f -> p h f", h=2), in_=o_sb[:]
            )

    return nc


_CACHE: dict = {}

# test.py sets these for profiling; harness never touches them.
LAST_RESULTS = None


def _trace_kwargs():
    if os.environ.get("BASS_KERNEL_TRACE") == "1":
        cores = os.environ.get("BASS_KERNEL_TRACE_CORES", "0")
        return dict(trace=True, trace_cores=[int(c) for c in cores.split(",")])
    return {}


def kernel(x: np.ndarray, W: np.ndarray, R: np.ndarray) -> np.ndarray:
    global LAST_RESULTS
    x = np.asarray(x, dtype=np.float32)
    W = np.asarray(W, dtype=np.float32)
    R = np.asarray(R, dtype=np.float32)

    # softmax over n (65K elements -- host)
    Rm = R.max(axis=0, keepdims=True)
    e = np.exp(R - Rm)
    Rs = (e / e.sum(axis=0, keepdims=True)).astype(np.float32)

    # upload layouts: x^T as [n, i, B], W i-major as [n, i, k, d], Rs
    # un-broadcast [n, k]; all in the kernel's bf16 compute precision
    Wp = (
        np.ascontiguousarray(W.transpose(0, 3, 1, 2)).reshape(N, F_W).astype(NPBF16)
    )
    Rp = np.ascontiguousarray(Rs).astype(NPBF16)  # [N, K]
    Xp = np.ascontiguousarray(x.transpose(1, 2, 0)).reshape(N, F_X).astype(NPBF16)

    in_maps = []
    for c in range(NCORES):
        sl = slice(c * NL, (c + 1) * NL)
        in_maps.append({"xs": Xp[sl], "ws": Wp[sl], "rs": Rp[sl]})

    if "nc" not in _CACHE:
        _CACHE["nc"] = build_bass()
    nc = _CACHE["nc"]

    res = run_bass_kernel_spmd(
        nc, in_maps, core_ids=list(range(NCORES)), **_trace_kwargs()
    )
    LAST_RESULTS = res

    s = np.zeros((B, KD), np.float32)
    for r in res.results:
        s += r["out"].astype(np.float32)
    s = s.reshape(B, K, D)
    sq = np.sum(np.square(s), axis=-1, keepdims=True) + EPS
    v = (np.sqrt(sq) / (1.0 + sq)) * s
    return v.astype(np.float32)


if __name__ == "__main__":
    rng = np.random.default_rng(0)
    x = rng.standard_normal((B, N, I), dtype=np.float32)
    W = (rng.standard_normal((N, K, D, I), dtype=np.float32) * 0.05).astype(np.float32)
    R = rng.standard_normal((N, K), dtype=np.float32)
    out = kernel(x, W, R)
    print("out", out.shape, out.dtype, float(np.abs(out).mean()))


# revision 26
# speedup vs baseline: 1.1182x; 1.0404x over previous
"""Capsule-routing kernel (einsum bni,nkdi,nk->bkd + squash) on 8 trn2 cores.

Sharding: over the contraction axis n (2048 -> 256 per core).  Each core
reads only its slice of x and W -- every input byte is read exactly once
machine-wide.  Each core emits a partial s[b,(k,d)] over its n-slice; the
host sums the 8 partials and applies the tiny squash nonlinearity.

v2 changes vs the 40.2us baseline (trace-driven):
  - W is shipped as int8 with a per-(n,k) host-computed scale a_w folded
    into the softmax(R) multiplier: wb = int8(W) * bf16(Rs*a_w).  Halves
    the dominant HBM stream (4MB -> 2MB/core); measured end-to-end rel
    err ~0.8% (gate 2e-2).
  - ALL input DMAs ride ONE HWDGE queue (sync) in explicit arrival
    order: rs, x0a, W0c0, x0b, W0c1, W0c2, x1, W1c0..2.  The old 2-queue
    layout round-robined x behind W, landing x0 at t=18.5us and stalling
    the first matmul.  Single-queue FIFO lands x0a ~4us in.
  - Rs is uploaded un-broadcast [n, k] (16KB not 256KB); the scale op
    broadcasts over (i, d) with stride-0 AP dims.
  - PE warm-up: ~36 dummy 128x128 matmuls on the identity tile right
    after the preamble keep the PE HAM busy so it unthrottles to 2.4GHz
    (K=8/8) before the first real matmul (trace showed the first ~14
    real matmuls ran at 1.2GHz).
  - Output DMAs go SWDGE (gpsimd, per-DMA semaphores) because the 10
    input DMAs exhaust the 8 HWDGE DMAHW lanes and a reused lane would
    add a second sem-wait (illegal in this walrus build).
  - acc0 (B rows 0..127) finishes 3 matmuls early (tail reordered
    h0,h0,h1,h1) so its PSUM evac + output DMA overlap acc1's tail.
  - Tile's kernel-sem range narrowed (teardown probe): the NEFF epilogue
    clears every semaphore one instruction at a time (~6.6us!); if the
    clear range follows the declared range this shrinks it.

The walrus build in this container accepts at most ONE sync-wait per
instruction.  Consequences handled here:
  - tiny DVE "toucher" ops absorb the rs/x DMA completions into DVE
    program order (so matmuls and scale ops carry at most one wait)
  - W-chunk scale ops carry their chunk's DMA wait directly (their other
    operand rs is already DVE-ordered)
  - input DMAs may reuse DMAHW lanes (their only wait); output DMAs are
    SWDGE so their single wait is the evac dependency
  - Tile's multi-wait kernel-tail drain is monkeypatched into a chain of
    single-wait drains
"""

import os
import sys

import numpy as np

if "/opt/trn_rl_repo" not in sys.path:
    sys.path.insert(0, "/opt/trn_rl_repo")

import bass_rust as _bass_rust
import concourse.bass as bass
import concourse.mybir as mybir
import ml_dtypes
from concourse.bass_utils import run_bass_kernel_spmd
from concourse.masks import make_identity
from concourse.tile import TileContext

NCORES = 8
B, N, I = 256, 2048, 16
K, D = 32, 16
NL = N // NCORES  # 256 n-values per core
KD = K * D  # 512
F_W = I * K * D  # 8192   (i-major W layout)
F_X = I * B  # 4096      (x^T layout: [n, i, B])
EPS = 1e-7

FP32 = mybir.dt.float32
BF16 = mybir.dt.bfloat16
INT8 = mybir.dt.int8
NPBF16 = ml_dtypes.bfloat16

# W DMA chunk boundaries in units of i, per 128-partition tile.  Only 6
# W DMAs total: the SWDGE engine has 8 semaphores and dispatch #9+ blocks
# until an earlier DMA fully completes (measured ~1.2us stalls).  Scale
# ops run at sub-chunk granularity (SCALE_SPLIT) so matmuls stream before
# a whole DMA chunk is scaled.
WCHUNKS0 = [(0, 6), (6, 11), (11, 16)]
WCHUNKS1 = [(0, 6), (6, 12), (12, 16)]

# Split Tile's multi-wait kernel-tail drain into a chain of single-wait
# drains (program order on the sync sequencer makes the chain equivalent).
if not getattr(TileContext, "_split_drain_patched", False):

    def _split_drain_and_barrier(self, tick_clock, wait_clock):
        gc = tick_clock.global_clock
        vals = list(gc)
        for j, v in enumerate(vals):
            if v > 0:
                sub = [0] * len(vals)
                sub[j] = v
                d = self.nc.sync.drain()
                wait_clock.add_sem_waits(
                    d.ins,
                    _bass_rust.ScopedClock({None: _bass_rust.VectorClock(sub)}),
                )
        self.nc.all_engine_barrier()
        assert self.sems is not None
        popped = self.nc._tile_sem_poison_stack.pop()
        assert popped is self._sem_poison
        self.nc.clear_and_free_semaphores(list(self.sems.allocated().values()))

    TileContext._drain_and_barrier = _split_drain_and_barrier
    TileContext._split_drain_patched = True


def build_bass() -> bass.Bass:
    nc = bass.Bass()
    x_d = nc.dram_tensor("xs", [NL, F_X], BF16, kind="ExternalInput")
    w_d = nc.dram_tensor("ws", [NL, F_W], BF16, kind="ExternalInput")
    r_d = nc.dram_tensor("rs", [NL, K], BF16, kind="ExternalInput")
    o_d = nc.dram_tensor("out", [B, KD], BF16, kind="ExternalOutput")

    with TileContext(nc) as tc:
        with (
            tc.tile_pool(name="big", bufs=1) as big,
            tc.tile_pool(name="ps_warm", bufs=1, space="PSUM") as ps_warm,
            tc.tile_pool(name="ps_acc", bufs=1, space="PSUM") as ps_acc,
        ):
            rs_kd = big.tile([128, 2 * K], BF16, tag="rs_kd")
            rse = big.tile([128, 2 * KD], BF16, tag="rse")
            xb = [big.tile([128, F_X], BF16, tag=f"x{t}", name=f"x{t}") for t in range(2)]
            wc = [big.tile([128, F_W], BF16, tag=f"w{t}", name=f"w{t}") for t in range(2)]
            wb = [big.tile([128, F_W], BF16, tag=f"wb{t}", name=f"wb{t}") for t in range(2)]

            # ---- input DMAs: W rides the gpsimd SWDGE queue (6 DMAs --
            # the SWDGE has only 8 semaphores and dispatch #9+ stalls on
            # ring reuse) in consumption order; rs + x ride the sync
            # HWDGE queue, which trickles alongside (the SDMA arbiter
            # favors the SWDGE queue heavily). ----
            identb = big.tile([128, 128], BF16, tag="identb")
            make_identity(nc, identb)

            def dma_w(t, chunk):
                i0, i1 = chunk
                nc.gpsimd.dma_start(
                    out=wc[t][:, i0 * KD : i1 * KD],
                    in_=w_d[t * 128 : (t + 1) * 128, i0 * KD : i1 * KD],
                )

            nc.sync.dma_start(
                out=rs_kd[:], in_=r_d.rearrange("(t p) k -> p t k", t=2)
            )
            nc.sync.dma_start(
                out=xb[0][:, : 8 * B], in_=x_d[0:128, : 8 * B]
            )
            nc.sync.dma_start(
                out=xb[0][:, 8 * B :], in_=x_d[0:128, 8 * B :]
            )
            nc.sync.dma_start(out=xb[1][:], in_=x_d[128:256, :])
            for chunk in WCHUNKS0:
                dma_w(0, chunk)
            for chunk in WCHUNKS1:
                dma_w(1, chunk)

            # ---- PE warm-up: a single transpose absorbs the gpsimd
            # identity dep into PE order.  No dummy-matmul burst: the
            # burst coincided with a P0 power downclock (matmul stream
            # 216 -> 259 ns) in measurement. ----
            warm_tp = ps_warm.tile([128, 128], BF16, tag="warmtp")
            nc.tensor.transpose(warm_tp[:], identb[:], identb[:])

            # ---- DVE pipeline in arrival order: touchers (absorb rs/x
            # DMA completions) interleaved with per-chunk scale ops ----
            def touch(name, src):
                tt = big.tile([128, 1], BF16, tag=f"touch_{name}")
                nc.vector.tensor_copy(tt[:], src)

            def scale_range(t, i0, i1):
                ni = i1 - i0
                sl_in = wc[t][:, i0 * KD : i1 * KD].rearrange(
                    "p (i f) -> p i f", f=KD
                )
                sl_out = wb[t][:, i0 * KD : i1 * KD].rearrange(
                    "p (i f) -> p i f", f=KD
                )
                r_sl = rse[:, t * KD : (t + 1) * KD]
                r_b = bass.AP(
                    tensor=r_sl.tensor,
                    offset=r_sl.offset,
                    ap=[r_sl.ap[0], [0, ni], [1, KD]],
                )
                nc.vector.tensor_mul(sl_out, sl_in, r_b)

            def scale(t, chunk):
                # toucher first: absorbs the chunk's DMA completion into
                # DVE program order, so each scale op's only sem wait is
                # the DVE counter (covers the rse dep too -- same sem).
                # The chunk is scaled in two halves so matmuls can start
                # on the first half while the second is still scaling.
                i0, i1 = chunk
                touch(f"w{t}_{i0}", wc[t][:, i0 * KD : i0 * KD + 1])
                mid = (i0 + i1 + 1) // 2
                scale_range(t, i0, mid)
                if mid < i1:
                    scale_range(t, mid, i1)

            touch("rs", rs_kd[:, 0:1])
            # broadcast Rs over d on-device, once (128K elems): rse[p,
            # (t k d)] = rs_kd[p, (t k)]; the per-chunk scale ops then use
            # the dense stride-1 form that keeps DVE in 2x perf mode
            src = rs_kd[:]
            r_src = bass.AP(
                tensor=src.tensor,
                offset=src.offset,
                ap=[src.ap[0], [1, 2 * K], [0, D]],
            )
            nc.vector.tensor_copy(rse[:].rearrange("p (k d) -> p k d", d=D), r_src)
            scale(0, WCHUNKS0[0])
            touch("x0a", xb[0][:, 0:1])
            scale(0, WCHUNKS0[1])
            touch("x0b", xb[0][:, 8 * B : 8 * B + 1])
            scale(0, WCHUNKS0[2])
            scale(1, WCHUNKS1[0])
            touch("x1", xb[1][:, 0:1])
            scale(1, WCHUNKS1[1])
            scale(1, WCHUNKS1[2])

            # ---- main matmuls ----
            # acc_h[b, (k d)] += xb[t][:, (i, h-half)]^T @ wb[t][:, i-slice].
            # Tail reordered h0,h0,h1,h1 so acc0 finalizes early and its
            # evac + output DMA overlap acc1's last matmuls.
            accs = [
                ps_acc.tile([128, KD], FP32, tag=f"acc{h}", name=f"acc{h}")
                for h in range(2)
            ]

            def mm(t, i, h, start, stop):
                rhs = wb[t][:, i * KD : (i + 1) * KD]
                lhsT = xb[t][:, i * B + h * 128 : i * B + (h + 1) * 128]
                nc.tensor.matmul(accs[h][:], lhsT, rhs, start=start, stop=stop)

            # tail reordered: the last W chunk (t=1, i 12..15) runs all
            # B-half-0 matmuls first, so acc0's evac + output DMA overlap
            # the B-half-1 matmuls
            for t in range(2):
                for i in range(I):
                    if t == 1 and i >= 12:
                        continue
                    first = t == 0 and i == 0
                    mm(t, i, 0, first, False)
                    mm(t, i, 1, first, False)
            for h in range(2):
                for i in range(12, I):
                    mm(1, i, h, False, i == I - 1)

            # ---- output: PSUM -> SBUF bf16 on DVE, two HWDGE out DMAs
            # on fresh DMAHW lanes (sync carried 4 input DMAs) ----
            o_sb = big.tile([128, 2 * KD], BF16, tag="osb")
            for h in range(2):
                nc.vector.tensor_copy(o_sb[:, h * KD : (h + 1) * KD], accs[h][:])
                nc.sync.dma_start(
                    out=o_d[h * 128 : (h + 1) * 128, :],
                    in_=o_sb[:, h * KD : (h + 1) * KD],
                )

    return nc


_CACHE: dict = {}

# test.py sets these for profiling; harness never touches them.
LAST_RESULTS = None


def _trace_kwargs():
    if os.environ.get("BASS_KERNEL_TRACE") == "1":
        cores = os.environ.get("BASS_KERNEL_TRACE_CORES", "0")
        return dict(trace=True, trace_cores=[int(c) for c in cores.split(",")])
    return {}


def kernel(x: np.ndarray, W: np.ndarray, R: np.ndarray) -> np.ndarray:
    global LAST_RESULTS
    x = np.asarray(x, dtype=np.float32)
    W = np.asarray(W, dtype=np.float32)
    R = np.asarray(R, dtype=np.float32)

    # softmax over n (65K elements -- host)
    Rm = R.max(axis=0, keepdims=True)
    e = np.exp(R - Rm)
    Rs = (e / e.sum(axis=0, keepdims=True)).astype(np.float32)

    # upload layouts: x^T as [n, i, B], W i-major as [n, i, k, d], Rs
    # un-broadcast [n, k]; all in the kernel's bf16 compute precision
    Wp = (
        np.ascontiguousarray(W.transpose(0, 3, 1, 2)).reshape(N, F_W).astype(NPBF16)
    )
    Rp = np.ascontiguousarray(Rs).astype(NPBF16)  # [N, K]
    Xp = np.ascontiguousarray(x.transpose(1, 2, 0)).reshape(N, F_X).astype(NPBF16)

    in_maps = []
    for c in range(NCORES):
        sl = slice(c * NL, (c + 1) * NL)
        in_maps.append({"xs": Xp[sl], "ws": Wp[sl], "rs": Rp[sl]})

    if "nc" not in _CACHE:
        _CACHE["nc"] = build_bass()
    nc = _CACHE["nc"]

    res = run_bass_kernel_spmd(
        nc, in_maps, core_ids=list(range(NCORES)), **_trace_kwargs()
    )
    LAST_RESULTS = res

    s = np.zeros((B, KD), np.float32)
    for r in res.results:
        s += r["out"].astype(np.float32)
    s = s.reshape(B, K, D)
    sq = np.sum(np.square(s), axis=-1, keepdims=True) + EPS
    v = (np.sqrt(sq) / (1.0 + sq)) * s
    return v.astype(np.float32)


if __name__ == "__main__":
    rng = np.random.default_rng(0)
    x = rng.standard_normal((B, N, I), dtype=np.float32)
    W = (rng.standard_normal((N, K, D, I), dtype=np.float32) * 0.05).astype(np.float32)
    R = rng.standard_normal((N, K), dtype=np.float32)
    out = kernel(x, W, R)
    print("out", out.shape, out.dtype, float(np.abs(out).mean()))


# revision 27
# speedup vs baseline: 1.1624x; 1.0395x over previous
"""Capsule-routing kernel (einsum bni,nkdi,nk->bkd + squash) on 8 trn2 cores.

Sharding: over the contraction axis n (2048 -> 256 per core).  Each core
reads only its slice of x and W -- every input byte is read exactly once
machine-wide.  Each core emits a partial s[b,(k,d)] over its n-slice; the
host sums the 8 partials and applies the tiny squash nonlinearity.

Precision: bf16 matmuls with fp32 PSUM accumulation; partial outputs in
bf16 (summed in fp32 on host).  Measured Frobenius rel err ~3.9e-3.

Structure (trace-driven, vs the 40.2us v1 baseline):
  - W slices ride the gpsimd SWDGE queue (the SDMA arbiter drains it far
    ahead of HWDGE queues) in consumption order, 4 chunks per 128-row
    tile sized [6,6,3,1] i's: the tail chunk is ONE i-slice, so the
    chain after the last HBM byte is just sem-receipt (~0.9us) + a small
    scale + 2 matmuls, instead of a 4-i chunk's worth.  8 SWDGE DMAs
    total -- at most 8, since the SWDGE has 8 sems and dispatch #9+
    stalls until an earlier DMA completes.
  - rs + x ride the sync HWDGE queue, which trickles alongside.
  - DVE ops are emitted in ARRIVAL order: toucher+scale per tile-0
    chunk, then the x1 toucher, then tile-1's.  (v1 interleaved the
    scale ops across tiles, so tile-0's second scale sat behind tile-1's
    first DMA -- mid-stream matmuls stalled ~2-3us on that.)
  - Matmul tail is h-split over the last two chunks: acc0 (B rows
    0..127) finalizes ~6 matmuls early and its PSUM evac + output DMA
    overlap acc1's tail matmuls.
  - PE warm-up is a single transpose (absorbs the gpsimd identity dep
    into PE order); no dummy-matmul burst.

The walrus build in this container accepts at most ONE sync-wait per
instruction.  Consequences handled here:
  - tiny DVE "toucher" ops absorb each DMA completion into DVE program
    order before real consumers run (so no op carries DMA + DVE waits)
  - HWDGE DMA count kept <= 8 so the output DMAs land on fresh DMAHW
    lanes (a lane-reuse wait on top of the data wait would be illegal)
  - Tile's multi-wait kernel-tail drain is monkeypatched into a chain of
    single-wait drains
"""

import os
import sys

import numpy as np

if "/opt/trn_rl_repo" not in sys.path:
    sys.path.insert(0, "/opt/trn_rl_repo")

import bass_rust as _bass_rust
import concourse.bass as bass
import concourse.mybir as mybir
import ml_dtypes
from concourse.bass_utils import run_bass_kernel_spmd
from concourse.masks import make_identity
from concourse.tile import TileContext

NCORES = 8
B, N, I = 256, 2048, 16
K, D = 32, 16
NL = N // NCORES  # 256 n-values per core
KD = K * D  # 512
F_W = I * K * D  # 8192   (i-major W layout)
F_X = I * B  # 4096      (x^T layout: [n, i, B])
EPS = 1e-7

FP32 = mybir.dt.float32
BF16 = mybir.dt.bfloat16
NPBF16 = ml_dtypes.bfloat16

# W DMA chunk boundaries in units of i, per 128-partition tile
WCHUNKS = [(0, 6), (6, 12), (12, 15), (15, 16)]

# Split Tile's multi-wait kernel-tail drain into a chain of single-wait
# drains (program order on the sync sequencer makes the chain equivalent).
if not getattr(TileContext, "_split_drain_patched", False):

    def _split_drain_and_barrier(self, tick_clock, wait_clock):
        gc = tick_clock.global_clock
        vals = list(gc)
        for j, v in enumerate(vals):
            if v > 0:
                sub = [0] * len(vals)
                sub[j] = v
                d = self.nc.sync.drain()
                wait_clock.add_sem_waits(
                    d.ins,
                    _bass_rust.ScopedClock({None: _bass_rust.VectorClock(sub)}),
                )
        self.nc.all_engine_barrier()
        assert self.sems is not None
        popped = self.nc._tile_sem_poison_stack.pop()
        assert popped is self._sem_poison
        self.nc.clear_and_free_semaphores(list(self.sems.allocated().values()))

    TileContext._drain_and_barrier = _split_drain_and_barrier
    TileContext._split_drain_patched = True


def build_bass() -> bass.Bass:
    nc = bass.Bass()
    x_d = nc.dram_tensor("xs", [NL, F_X], BF16, kind="ExternalInput")
    w_d = nc.dram_tensor("ws", [NL, F_W], BF16, kind="ExternalInput")
    r_d = nc.dram_tensor("rs", [NL, KD], BF16, kind="ExternalInput")
    o_d = nc.dram_tensor("out", [B, KD], BF16, kind="ExternalOutput")

    with TileContext(nc) as tc:
        with (
            tc.tile_pool(name="big", bufs=1) as big,
            tc.tile_pool(name="ps_warm", bufs=1, space="PSUM") as ps_warm,
            tc.tile_pool(name="ps_acc", bufs=1, space="PSUM") as ps_acc,
        ):
            rs_kd = big.tile([128, 2 * KD], BF16, tag="rs_kd")
            xb = [big.tile([128, F_X], BF16, tag=f"x{t}", name=f"x{t}") for t in range(2)]
            ws = [big.tile([128, F_W], BF16, tag=f"w{t}", name=f"w{t}") for t in range(2)]
            wb = [big.tile([128, F_W], BF16, tag=f"wb{t}", name=f"wb{t}") for t in range(2)]

            # ---- input DMAs ----
            nc.sync.dma_start(
                out=rs_kd[:], in_=r_d.rearrange("(t p) f -> p t f", t=2)
            )
            for t in range(2):
                nc.sync.dma_start(
                    out=xb[t][:], in_=x_d[t * 128 : (t + 1) * 128, :]
                )
            for t in range(2):
                for i0, i1 in WCHUNKS:
                    nc.gpsimd.dma_start(
                        out=ws[t][:, i0 * KD : i1 * KD],
                        in_=w_d[t * 128 : (t + 1) * 128, i0 * KD : i1 * KD],
                    )

            # identity for the PE warm-up (gpsimd ops AFTER the W
            # dispatches, so the W stream's doorbells ring first)
            identb = big.tile([128, 128], BF16, tag="identb")
            make_identity(nc, identb)
            warm_ps = ps_warm.tile([128, 128], BF16, tag="warmps")
            nc.tensor.transpose(warm_ps[:], identb[:], identb[:])

            # ---- DVE pipeline in arrival order: touchers absorb DMA
            # completions; one scale op per W chunk ----
            def touch(name, src):
                tt = big.tile([128, 1], BF16, tag=f"touch_{name}")
                nc.vector.tensor_copy(tt[:], src)

            def scale(t, chunk):
                i0, i1 = chunk
                ni = i1 - i0
                touch(f"w{t}_{i0}", ws[t][:, i0 * KD : i0 * KD + 1])
                sl_in = ws[t][:, i0 * KD : i1 * KD].rearrange(
                    "p (i f) -> p i f", f=KD
                )
                sl_out = wb[t][:, i0 * KD : i1 * KD].rearrange(
                    "p (i f) -> p i f", f=KD
                )
                r_sl = rs_kd[:, t * KD : (t + 1) * KD]
                r_b = bass.AP(
                    tensor=r_sl.tensor,
                    offset=r_sl.offset,
                    ap=[r_sl.ap[0], [0, ni], [1, KD]],
                )
                nc.vector.tensor_mul(sl_out, sl_in, r_b)

            touch("rs", rs_kd[:, 0:1])
            touch("x0", xb[0][:, 0:1])
            for chunk in WCHUNKS:
                scale(0, chunk)
            touch("x1", xb[1][:, 0:1])
            for chunk in WCHUNKS:
                scale(1, chunk)

            # ---- main matmuls ----
            # acc_h[b, (k d)] += xb[t][:, (i, h-half)]^T @ wb[t][:, i-slice].
            # Tail (t=1, i>=12) is h-split: acc0 finalizes 6 matmuls early
            # so its evac + output DMA overlap acc1's tail matmuls.
            accs = [
                ps_acc.tile([128, KD], FP32, tag=f"acc{h}", name=f"acc{h}")
                for h in range(2)
            ]

            def mm(t, i, h, start, stop):
                rhs = wb[t][:, i * KD : (i + 1) * KD]
                lhsT = xb[t][:, i * B + h * 128 : i * B + (h + 1) * 128]
                nc.tensor.matmul(accs[h][:], lhsT, rhs, start=start, stop=stop)

            for t in range(2):
                for i in range(I):
                    if t == 1 and i >= 12:
                        continue
                    first = t == 0 and i == 0
                    mm(t, i, 0, first, False)
                    mm(t, i, 1, first, False)
            for h in range(2):
                for i in range(12, I):
                    mm(1, i, h, False, i == I - 1)

            # ---- output: PSUM -> SBUF bf16 on DVE, two HWDGE out DMAs
            # on fresh DMAHW lanes (sync carried only 3 input DMAs) ----
            o_sb = big.tile([128, 2 * KD], BF16, tag="osb")
            for h in range(2):
                nc.vector.tensor_copy(o_sb[:, h * KD : (h + 1) * KD], accs[h][:])
                nc.sync.dma_start(
                    out=o_d[h * 128 : (h + 1) * 128, :],
                    in_=o_sb[:, h * KD : (h + 1) * KD],
                )

    return nc


_CACHE: dict = {}

# test.py sets these for profiling; harness never touches them.
LAST_RESULTS = None


def _trace_kwargs():
    if os.environ.get("BASS_KERNEL_TRACE") == "1":
        cores = os.environ.get("BASS_KERNEL_TRACE_CORES", "0")
        return dict(trace=True, trace_cores=[int(c) for c in cores.split(",")])
    return {}


def kernel(x: np.ndarray, W: np.ndarray, R: np.ndarray) -> np.ndarray:
    global LAST_RESULTS
    x = np.asarray(x, dtype=np.float32)
    W = np.asarray(W, dtype=np.float32)
    R = np.asarray(R, dtype=np.float32)

    # softmax over n (65K elements -- host)
    Rm = R.max(axis=0, keepdims=True)
    e = np.exp(R - Rm)
    Rs = (e / e.sum(axis=0, keepdims=True)).astype(np.float32)

    # upload layouts: x^T as [n, i, B], W i-major as [n, i, k, d], Rs
    # pre-broadcast over d as [n, (k d)]; all in the kernel's bf16
    # compute precision
    Xp = np.ascontiguousarray(x.transpose(1, 2, 0)).reshape(N, F_X).astype(NPBF16)
    Wp = np.ascontiguousarray(W.transpose(0, 3, 1, 2)).reshape(N, F_W).astype(NPBF16)
    Rp = np.ascontiguousarray(np.repeat(Rs, D, axis=1)).astype(NPBF16)
    in_maps = []
    for c in range(NCORES):
        sl = slice(c * NL, (c + 1) * NL)
        in_maps.append({"xs": Xp[sl], "ws": Wp[sl], "rs": Rp[sl]})

    if "nc" not in _CACHE:
        _CACHE["nc"] = build_bass()
    nc = _CACHE["nc"]

    res = run_bass_kernel_spmd(
        nc, in_maps, core_ids=list(range(NCORES)), **_trace_kwargs()
    )
    LAST_RESULTS = res

    s = np.zeros((B, KD), np.float32)
    for r in res.results:
        s += r["out"].astype(np.float32)
    s = s.reshape(B, K, D)
    sq = np.sum(np.square(s), axis=-1, keepdims=True) + EPS
    v = (np.sqrt(sq) / (1.0 + sq)) * s
    return v.astype(np.float32)


if __name__ == "__main__":
    rng = np.random.default_rng(0)
    x = rng.standard_normal((B, N, I), dtype=np.float32)
    W = (rng.standard_normal((N, K, D, I), dtype=np.float32) * 0.05).astype(np.float32)
    R = rng.standard_normal((N, K), dtype=np.float32)
    out = kernel(x, W, R)
    print("out", out.shape, out.dtype, float(np.abs(out).mean()))


# revision 30
# speedup vs baseline: 1.1829x; 1.0177x over previous
"""Capsule-routing kernel (einsum bni,nkdi,nk->bkd + squash) on 8 trn2 cores.

Sharding: over the contraction axis n (2048 -> 256 per core).  Each core
reads only its slice of x and W -- every input byte is read exactly once
machine-wide.  Each core emits a partial s[b,(k,d)] over its n-slice; the
host sums the 8 partials and applies the tiny squash nonlinearity.

Precision: bf16 matmuls with fp32 PSUM accumulation; partial outputs in
bf16 (summed in fp32 on host).  Measured Frobenius rel err ~3.9e-3.

Structure (trace-driven, vs the 40.2us v1 baseline):
  - W slices ride the gpsimd SWDGE queue (the SDMA arbiter drains it far
    ahead of HWDGE queues) in consumption order, 4 chunks per 128-row
    tile sized [6,6,3,1] i's: the tail chunk is ONE i-slice, so the
    chain after the last HBM byte is just sem-receipt (~0.9us) + a small
    scale + 2 matmuls, instead of a 4-i chunk's worth.  8 SWDGE DMAs
    total -- at most 8, since the SWDGE has 8 sems and dispatch #9+
    stalls until an earlier DMA completes.
  - rs + x ride the sync HWDGE queue, which trickles alongside.
  - DVE ops are emitted in ARRIVAL order: toucher+scale per tile-0
    chunk, then the x1 toucher, then tile-1's.  (v1 interleaved the
    scale ops across tiles, so tile-0's second scale sat behind tile-1's
    first DMA -- mid-stream matmuls stalled ~2-3us on that.)
  - Matmul tail is h-split over the last two chunks: acc0 (B rows
    0..127) finalizes ~6 matmuls early and its PSUM evac + output DMA
    overlap acc1's tail matmuls.
  - PE warm-up is a single transpose (absorbs the gpsimd identity dep
    into PE order); no dummy-matmul burst.

The walrus build in this container accepts at most ONE sync-wait per
instruction.  Consequences handled here:
  - tiny DVE "toucher" ops absorb each DMA completion into DVE program
    order before real consumers run (so no op carries DMA + DVE waits)
  - HWDGE DMA count kept <= 8 so the output DMAs land on fresh DMAHW
    lanes (a lane-reuse wait on top of the data wait would be illegal)
  - Tile's multi-wait kernel-tail drain is monkeypatched into a chain of
    single-wait drains
"""

import os
import sys

import numpy as np

if "/opt/trn_rl_repo" not in sys.path:
    sys.path.insert(0, "/opt/trn_rl_repo")

import bass_rust as _bass_rust
import concourse.bass as bass
import concourse.mybir as mybir
import ml_dtypes
from concourse.bass_utils import run_bass_kernel_spmd
from concourse.masks import make_identity
from concourse.tile import TileContext

NCORES = 8
B, N, I = 256, 2048, 16
K, D = 32, 16
NL = N // NCORES  # 256 n-values per core
KD = K * D  # 512
F_W = I * K * D  # 8192   (i-major W layout)
F_X = I * B  # 4096      (x^T layout: [n, i, B])
EPS = 1e-7

FP32 = mybir.dt.float32
BF16 = mybir.dt.bfloat16
NPBF16 = ml_dtypes.bfloat16

# W DMA chunk boundaries in units of i, per tile, and the scale-op
# sub-splits within each chunk (sub-ops after one toucher carry no sem
# waits, so scale granularity is decoupled from the 8-SWDGE-DMA budget)
WCHUNKS0 = [(0, 4), (4, 10), (10, 16)]
WCHUNKS1 = [(0, 8), (8, 14), (14, 16)]
SUBSPLIT = {
    (0, 4): [(0, 2), (2, 4)],
    (4, 10): [(4, 7), (7, 10)],
    (10, 16): [(10, 13), (13, 16)],
    (0, 8): [(0, 3), (3, 6), (6, 8)],
    (8, 14): [(8, 11), (11, 14)],
    (14, 16): [(14, 16)],
}

# Split Tile's multi-wait kernel-tail drain into a chain of single-wait
# drains (program order on the sync sequencer makes the chain equivalent).
if not getattr(TileContext, "_split_drain_patched", False):

    def _split_drain_and_barrier(self, tick_clock, wait_clock):
        gc = tick_clock.global_clock
        vals = list(gc)
        for j, v in enumerate(vals):
            if v > 0:
                sub = [0] * len(vals)
                sub[j] = v
                d = self.nc.sync.drain()
                wait_clock.add_sem_waits(
                    d.ins,
                    _bass_rust.ScopedClock({None: _bass_rust.VectorClock(sub)}),
                )
        self.nc.all_engine_barrier()
        assert self.sems is not None
        popped = self.nc._tile_sem_poison_stack.pop()
        assert popped is self._sem_poison
        self.nc.clear_and_free_semaphores(list(self.sems.allocated().values()))

    TileContext._drain_and_barrier = _split_drain_and_barrier
    TileContext._split_drain_patched = True


def build_bass() -> bass.Bass:
    nc = bass.Bass()
    x_d = nc.dram_tensor("xs", [NL, F_X], BF16, kind="ExternalInput")
    w_d = nc.dram_tensor("ws", [NL, F_W], BF16, kind="ExternalInput")
    r_d = nc.dram_tensor("rs", [NL, KD], BF16, kind="ExternalInput")
    o_d = nc.dram_tensor("out", [B, KD], BF16, kind="ExternalOutput")

    with TileContext(nc) as tc:
        with (
            tc.tile_pool(name="big", bufs=1) as big,
            tc.tile_pool(name="ps_warm", bufs=1, space="PSUM") as ps_warm,
            tc.tile_pool(name="ps_acc", bufs=1, space="PSUM") as ps_acc,
        ):
            rs_kd = big.tile([128, 2 * KD], BF16, tag="rs_kd")
            xb = [big.tile([128, F_X], BF16, tag=f"x{t}", name=f"x{t}") for t in range(2)]
            ws = [big.tile([128, F_W], BF16, tag=f"w{t}", name=f"w{t}") for t in range(2)]
            wb = [big.tile([128, F_W], BF16, tag=f"wb{t}", name=f"wb{t}") for t in range(2)]

            # ---- input DMAs.  The critical early stream (x-tile-0 +
            # all W) rides the gpsimd SWDGE queue in consumption order
            # (8 DMAs, the SWDGE sem budget); rs + x-tile-1 trickle on
            # the sync HWDGE queue (x1 is only needed ~mid-kernel). ----
            nc.sync.dma_start(
                out=rs_kd[:], in_=r_d.rearrange("(t p) f -> p t f", t=2)
            )
            nc.sync.dma_start(out=xb[1][:], in_=x_d[128:256, :])

            def dma_w(t, chunk):
                i0, i1 = chunk
                nc.gpsimd.dma_start(
                    out=ws[t][:, i0 * KD : i1 * KD],
                    in_=w_d[t * 128 : (t + 1) * 128, i0 * KD : i1 * KD],
                )

            nc.gpsimd.dma_start(
                out=xb[0][:, : 8 * B], in_=x_d[0:128, : 8 * B]
            )
            dma_w(0, WCHUNKS0[0])
            nc.gpsimd.dma_start(
                out=xb[0][:, 8 * B :], in_=x_d[0:128, 8 * B :]
            )
            dma_w(0, WCHUNKS0[1])
            dma_w(0, WCHUNKS0[2])
            for chunk in WCHUNKS1:
                dma_w(1, chunk)

            # identity for the PE warm-up (gpsimd ops AFTER the W
            # dispatches, so the W stream's doorbells ring first)
            identb = big.tile([128, 128], BF16, tag="identb")
            make_identity(nc, identb)
            warm_ps = ps_warm.tile([128, 128], BF16, tag="warmps")
            nc.tensor.transpose(warm_ps[:], identb[:], identb[:])

            # ---- DVE pipeline in arrival order: touchers absorb DMA
            # completions; one scale op per W chunk ----
            def touch(name, src):
                tt = big.tile([128, 1], BF16, tag=f"touch_{name}")
                nc.vector.tensor_copy(tt[:], src)

            def scale_range(t, i0, i1):
                ni = i1 - i0
                sl_in = ws[t][:, i0 * KD : i1 * KD].rearrange(
                    "p (i f) -> p i f", f=KD
                )
                sl_out = wb[t][:, i0 * KD : i1 * KD].rearrange(
                    "p (i f) -> p i f", f=KD
                )
                r_sl = rs_kd[:, t * KD : (t + 1) * KD]
                r_b = bass.AP(
                    tensor=r_sl.tensor,
                    offset=r_sl.offset,
                    ap=[r_sl.ap[0], [0, ni], [1, KD]],
                )
                nc.vector.tensor_mul(sl_out, sl_in, r_b)

            def scale(t, chunk):
                touch(f"w{t}_{chunk[0]}", ws[t][:, chunk[0] * KD : chunk[0] * KD + 1])
                for i0, i1 in SUBSPLIT[chunk]:
                    scale_range(t, i0, i1)

            touch("rs", rs_kd[:, 0:1])
            touch("x0a", xb[0][:, 0:1])
            scale(0, WCHUNKS0[0])
            touch("x0b", xb[0][:, 8 * B : 8 * B + 1])
            scale(0, WCHUNKS0[1])
            scale(0, WCHUNKS0[2])
            touch("x1", xb[1][:, 0:1])
            for chunk in WCHUNKS1:
                scale(1, chunk)

            # ---- main matmuls ----
            # acc_h[b, (k d)] += xb[t][:, (i, h-half)]^T @ wb[t][:, i-slice].
            # Tail (t=1, i>=12) is h-split: acc0 finalizes 6 matmuls early
            # so its evac + output DMA overlap acc1's tail matmuls.
            accs = [
                ps_acc.tile([128, KD], FP32, tag=f"acc{h}", name=f"acc{h}")
                for h in range(2)
            ]

            def mm(t, i, h, start, stop):
                rhs = wb[t][:, i * KD : (i + 1) * KD]
                lhsT = xb[t][:, i * B + h * 128 : i * B + (h + 1) * 128]
                nc.tensor.matmul(accs[h][:], lhsT, rhs, start=start, stop=stop)

            for t in range(2):
                for i in range(I):
                    if t == 1 and i >= 12:
                        continue
                    first = t == 0 and i == 0
                    mm(t, i, 0, first, False)
                    mm(t, i, 1, first, False)
            for h in range(2):
                for i in range(12, I):
                    mm(1, i, h, False, i == I - 1)

            # ---- output: PSUM -> SBUF bf16 on DVE, two HWDGE out DMAs
            # on fresh DMAHW lanes (sync carried only 3 input DMAs) ----
            o_sb = big.tile([128, 2 * KD], BF16, tag="osb")
            for h in range(2):
                nc.vector.tensor_copy(o_sb[:, h * KD : (h + 1) * KD], accs[h][:])
                nc.sync.dma_start(
                    out=o_d[h * 128 : (h + 1) * 128, :],
                    in_=o_sb[:, h * KD : (h + 1) * KD],
                )

    return nc


_CACHE: dict = {}

# test.py sets these for profiling; harness never touches them.
LAST_RESULTS = None


def _trace_kwargs():
    if os.environ.get("BASS_KERNEL_TRACE") == "1":
        cores = os.environ.get("BASS_KERNEL_TRACE_CORES", "0")
        return dict(trace=True, trace_cores=[int(c) for c in cores.split(",")])
    return {}


def kernel(x: np.ndarray, W: np.ndarray, R: np.ndarray) -> np.ndarray:
    global LAST_RESULTS
    x = np.asarray(x, dtype=np.float32)
    W = np.asarray(W, dtype=np.float32)
    R = np.asarray(R, dtype=np.float32)

    # softmax over n (65K elements -- host)
    Rm = R.max(axis=0, keepdims=True)
    e = np.exp(R - Rm)
    Rs = (e / e.sum(axis=0, keepdims=True)).astype(np.float32)

    # upload layouts: x^T as [n, i, B], W i-major as [n, i, k, d], Rs
    # pre-broadcast over d as [n, (k d)]; all in the kernel's bf16
    # compute precision
    Xp = np.ascontiguousarray(x.transpose(1, 2, 0)).reshape(N, F_X).astype(NPBF16)
    Wp = np.ascontiguousarray(W.transpose(0, 3, 1, 2)).reshape(N, F_W).astype(NPBF16)
    Rp = np.ascontiguousarray(np.repeat(Rs, D, axis=1)).astype(NPBF16)
    in_maps = []
    for c in range(NCORES):
        sl = slice(c * NL, (c + 1) * NL)
        in_maps.append({"xs": Xp[sl], "ws": Wp[sl], "rs": Rp[sl]})

    if "nc" not in _CACHE:
        _CACHE["nc"] = build_bass()
    nc = _CACHE["nc"]

    res = run_bass_kernel_spmd(
        nc, in_maps, core_ids=list(range(NCORES)), **_trace_kwargs()
    )
    LAST_RESULTS = res

    s = np.zeros((B, KD), np.float32)
    for r in res.results:
        s += r["out"].astype(np.float32)
    s = s.reshape(B, K, D)
    sq = np.sum(np.square(s), axis=-1, keepdims=True) + EPS
    v = (np.sqrt(sq) / (1.0 + sq)) * s
    return v.astype(np.float32)


if __name__ == "__main__":
    rng = np.random.default_rng(0)
    x = rng.standard_normal((B, N, I), dtype=np.float32)
    W = (rng.standard_normal((N, K, D, I), dtype=np.float32) * 0.05).astype(np.float32)
    R = rng.standard_normal((N, K), dtype=np.float32)
    out = kernel(x, W, R)
    print("out", out.shape, out.dtype, float(np.abs(out).mean()))


# revision 31
# speedup vs baseline: 1.1861x; 1.0027x over previous
"""Capsule-routing kernel (einsum bni,nkdi,nk->bkd + squash) on 8 trn2 cores.

Sharding: over the contraction axis n (2048 -> 256 per core).  Each core
reads only its slice of x and W -- every input byte is read exactly once
machine-wide.  Each core emits a partial s[b,(k,d)] over its n-slice; the
host sums the 8 partials and applies the tiny squash nonlinearity.

Precision: bf16 matmuls with fp32 PSUM accumulation; partial outputs in
bf16 (summed in fp32 on host).  Measured Frobenius rel err ~3.9e-3.

Structure (trace-driven, vs the 40.2us v1 baseline):
  - The critical stream -- x-tile-0 (split in two) followed by ALL W
    chunks -- rides the gpsimd SWDGE queue in consumption order: the
    SDMA arbiter drains that queue far ahead of HWDGE queues, so FIFO
    position there controls arrival.  Exactly 8 SWDGE DMAs (the SWDGE
    has 8 sems; dispatch #9+ stalls until an earlier DMA completes).
  - rs and x-tile-1 trickle on the sync HWDGE queue concurrently
    (~90-130 GB/s next to the SWDGE stream; x1 is only consumed from
    matmul #33 on, and lands mid-kernel).  Combined arrival ~330 GB/s.
  - W's tail chunk is 2 i-slices, so the chain after the last HBM byte
    is sem-receipt (~0.9us) + a small scale + a few matmuls.
  - Scale ops are sub-split within each W DMA chunk (after one toucher
    per chunk, sub-ops carry no sem waits) and emitted in ARRIVAL
    order.  (v1 interleaved the scale ops across tiles, so tile-0's
    second scale sat behind tile-1's first DMA -- mid-stream matmuls
    stalled ~2-3us on that.)
  - Matmul tail is h-split over the last chunks: acc0 (B rows 0..127)
    finalizes ~6 matmuls early and its PSUM evac + output DMA overlap
    acc1's tail matmuls.
  - PE warm-up is a single transpose (absorbs the gpsimd identity dep
    into PE order); no dummy-matmul burst.

The walrus build in this container accepts at most ONE sync-wait per
instruction.  Consequences handled here:
  - tiny DVE "toucher" ops absorb each DMA completion into DVE program
    order before real consumers run (so no op carries DMA + DVE waits)
  - HWDGE DMA count kept <= 8 so the output DMAs land on fresh DMAHW
    lanes (a lane-reuse wait on top of the data wait would be illegal)
  - Tile's multi-wait kernel-tail drain is monkeypatched into a chain of
    single-wait drains
"""

import os
import sys

import numpy as np

if "/opt/trn_rl_repo" not in sys.path:
    sys.path.insert(0, "/opt/trn_rl_repo")

import bass_rust as _bass_rust
import concourse.bass as bass
import concourse.mybir as mybir
import ml_dtypes
from concourse.bass_utils import run_bass_kernel_spmd
from concourse.masks import make_identity
from concourse.tile import TileContext

NCORES = 8
B, N, I = 256, 2048, 16
K, D = 32, 16
NL = N // NCORES  # 256 n-values per core
KD = K * D  # 512
F_W = I * K * D  # 8192   (i-major W layout)
F_X = I * B  # 4096      (x^T layout: [n, i, B])
EPS = 1e-7

FP32 = mybir.dt.float32
BF16 = mybir.dt.bfloat16
NPBF16 = ml_dtypes.bfloat16

# W DMA chunk boundaries in units of i, per tile, and the scale-op
# sub-splits within each chunk (sub-ops after one toucher carry no sem
# waits, so scale granularity is decoupled from the 8-SWDGE-DMA budget)
WCHUNKS0 = [(0, 4), (4, 10), (10, 16)]
WCHUNKS1 = [(0, 8), (8, 14), (14, 16)]
SUBSPLIT = {
    (0, 4): [(0, 2), (2, 4)],
    (4, 10): [(4, 7), (7, 10)],
    (10, 16): [(10, 13), (13, 16)],
    (0, 8): [(0, 3), (3, 6), (6, 8)],
    (8, 14): [(8, 11), (11, 14)],
    (14, 16): [(14, 16)],
}

# Split Tile's multi-wait kernel-tail drain into a chain of single-wait
# drains (program order on the sync sequencer makes the chain equivalent).
if not getattr(TileContext, "_split_drain_patched", False):

    def _split_drain_and_barrier(self, tick_clock, wait_clock):
        gc = tick_clock.global_clock
        vals = list(gc)
        for j, v in enumerate(vals):
            if v > 0:
                sub = [0] * len(vals)
                sub[j] = v
                d = self.nc.sync.drain()
                wait_clock.add_sem_waits(
                    d.ins,
                    _bass_rust.ScopedClock({None: _bass_rust.VectorClock(sub)}),
                )
        self.nc.all_engine_barrier()
        assert self.sems is not None
        popped = self.nc._tile_sem_poison_stack.pop()
        assert popped is self._sem_poison
        self.nc.clear_and_free_semaphores(list(self.sems.allocated().values()))

    TileContext._drain_and_barrier = _split_drain_and_barrier
    TileContext._split_drain_patched = True


def build_bass() -> bass.Bass:
    nc = bass.Bass()
    x_d = nc.dram_tensor("xs", [NL, F_X], BF16, kind="ExternalInput")
    w_d = nc.dram_tensor("ws", [NL, F_W], BF16, kind="ExternalInput")
    r_d = nc.dram_tensor("rs", [NL, KD], BF16, kind="ExternalInput")
    o_d = nc.dram_tensor("out", [B, KD], BF16, kind="ExternalOutput")

    with TileContext(nc) as tc:
        with (
            tc.tile_pool(name="big", bufs=1) as big,
            tc.tile_pool(name="ps_warm", bufs=1, space="PSUM") as ps_warm,
            tc.tile_pool(name="ps_acc", bufs=1, space="PSUM") as ps_acc,
        ):
            rs_kd = big.tile([128, 2 * KD], BF16, tag="rs_kd")
            xb = [big.tile([128, F_X], BF16, tag=f"x{t}", name=f"x{t}") for t in range(2)]
            ws = [big.tile([128, F_W], BF16, tag=f"w{t}", name=f"w{t}") for t in range(2)]
            wb = [big.tile([128, F_W], BF16, tag=f"wb{t}", name=f"wb{t}") for t in range(2)]

            # ---- input DMAs.  The critical early stream (x-tile-0 +
            # all W) rides the gpsimd SWDGE queue in consumption order
            # (8 DMAs, the SWDGE sem budget); rs + x-tile-1 trickle on
            # the sync HWDGE queue (x1 is only needed ~mid-kernel). ----
            nc.sync.dma_start(
                out=rs_kd[:], in_=r_d.rearrange("(t p) f -> p t f", t=2)
            )
            nc.sync.dma_start(out=xb[1][:], in_=x_d[128:256, :])

            def dma_w(t, chunk):
                i0, i1 = chunk
                nc.gpsimd.dma_start(
                    out=ws[t][:, i0 * KD : i1 * KD],
                    in_=w_d[t * 128 : (t + 1) * 128, i0 * KD : i1 * KD],
                )

            nc.gpsimd.dma_start(
                out=xb[0][:, : 8 * B], in_=x_d[0:128, : 8 * B]
            )
            dma_w(0, WCHUNKS0[0])
            nc.gpsimd.dma_start(
                out=xb[0][:, 8 * B :], in_=x_d[0:128, 8 * B :]
            )
            dma_w(0, WCHUNKS0[1])
            dma_w(0, WCHUNKS0[2])
            for chunk in WCHUNKS1:
                dma_w(1, chunk)

            # identity for the PE warm-up (gpsimd ops AFTER the W
            # dispatches, so the W stream's doorbells ring first)
            identb = big.tile([128, 128], BF16, tag="identb")
            make_identity(nc, identb)
            warm_ps = ps_warm.tile([128, 128], BF16, tag="warmps")
            nc.tensor.transpose(warm_ps[:], identb[:], identb[:])

            # ---- DVE pipeline in arrival order: touchers absorb DMA
            # completions; one scale op per W chunk ----
            def touch(name, src):
                tt = big.tile([128, 1], BF16, tag=f"touch_{name}")
                nc.vector.tensor_copy(tt[:], src)

            def scale_range(t, i0, i1):
                ni = i1 - i0
                sl_in = ws[t][:, i0 * KD : i1 * KD].rearrange(
                    "p (i f) -> p i f", f=KD
                )
                sl_out = wb[t][:, i0 * KD : i1 * KD].rearrange(
                    "p (i f) -> p i f", f=KD
                )
                r_sl = rs_kd[:, t * KD : (t + 1) * KD]
                r_b = bass.AP(
                    tensor=r_sl.tensor,
                    offset=r_sl.offset,
                    ap=[r_sl.ap[0], [0, ni], [1, KD]],
                )
                nc.vector.tensor_mul(sl_out, sl_in, r_b)

            def scale(t, chunk):
                touch(f"w{t}_{chunk[0]}", ws[t][:, chunk[0] * KD : chunk[0] * KD + 1])
                for i0, i1 in SUBSPLIT[chunk]:
                    scale_range(t, i0, i1)

            touch("rs", rs_kd[:, 0:1])
            touch("x0a", xb[0][:, 0:1])
            scale(0, WCHUNKS0[0])
            touch("x0b", xb[0][:, 8 * B : 8 * B + 1])
            scale(0, WCHUNKS0[1])
            scale(0, WCHUNKS0[2])
            touch("x1", xb[1][:, 0:1])
            for chunk in WCHUNKS1:
                scale(1, chunk)

            # ---- main matmuls ----
            # acc_h[b, (k d)] += xb[t][:, (i, h-half)]^T @ wb[t][:, i-slice].
            # Tail (t=1, i>=12) is h-split: acc0 finalizes 6 matmuls early
            # so its evac + output DMA overlap acc1's tail matmuls.
            accs = [
                ps_acc.tile([128, KD], FP32, tag=f"acc{h}", name=f"acc{h}")
                for h in range(2)
            ]

            def mm(t, i, h, start, stop):
                rhs = wb[t][:, i * KD : (i + 1) * KD]
                lhsT = xb[t][:, i * B + h * 128 : i * B + (h + 1) * 128]
                nc.tensor.matmul(accs[h][:], lhsT, rhs, start=start, stop=stop)

            for t in range(2):
                for i in range(I):
                    if t == 1 and i >= 12:
                        continue
                    first = t == 0 and i == 0
                    mm(t, i, 0, first, False)
                    mm(t, i, 1, first, False)
            for h in range(2):
                for i in range(12, I):
                    mm(1, i, h, False, i == I - 1)

            # ---- output: PSUM -> SBUF bf16 on DVE, two HWDGE out DMAs
            # on fresh DMAHW lanes (sync carried only 3 input DMAs) ----
            o_sb = big.tile([128, 2 * KD], BF16, tag="osb")
            for h in range(2):
                nc.vector.tensor_copy(o_sb[:, h * KD : (h + 1) * KD], accs[h][:])
                nc.sync.dma_start(
                    out=o_d[h * 128 : (h + 1) * 128, :],
                    in_=o_sb[:, h * KD : (h + 1) * KD],
                )

    return nc


_CACHE: dict = {}

# test.py sets these for profiling; harness never touches them.
LAST_RESULTS = None


def _trace_kwargs():
    if os.environ.get("BASS_KERNEL_TRACE") == "1":
        cores = os.environ.get("BASS_KERNEL_TRACE_CORES", "0")
        return dict(trace=True, trace_cores=[int(c) for c in cores.split(",")])
    return {}


def kernel(x: np.ndarray, W: np.ndarray, R: np.ndarray) -> np.ndarray:
    global LAST_RESULTS
    x = np.asarray(x, dtype=np.float32)
    W = np.asarray(W, dtype=np.float32)
    R = np.asarray(R, dtype=np.float32)

    # softmax over n (65K elements -- host)
    Rm = R.max(axis=0, keepdims=True)
    e = np.exp(R - Rm)
    Rs = (e / e.sum(axis=0, keepdims=True)).astype(np.float32)

    # upload layouts: x^T as [n, i, B], W i-major as [n, i, k, d], Rs
    # pre-broadcast over d as [n, (k d)]; all in the kernel's bf16
    # compute precision
    Xp = np.ascontiguousarray(x.transpose(1, 2, 0)).reshape(N, F_X).astype(NPBF16)
    Wp = np.ascontiguousarray(W.transpose(0, 3, 1, 2)).reshape(N, F_W).astype(NPBF16)
    Rp = np.ascontiguousarray(np.repeat(Rs, D, axis=1)).astype(NPBF16)
    in_maps = []
    for c in range(NCORES):
        sl = slice(c * NL, (c + 1) * NL)
        in_maps.append({"xs": Xp[sl], "ws": Wp[sl], "rs": Rp[sl]})

    if "nc" not in _CACHE:
        _CACHE["nc"] = build_bass()
    nc = _CACHE["nc"]

    res = run_bass_kernel_spmd(
        nc, in_maps, core_ids=list(range(NCORES)), **_trace_kwargs()
    )
    LAST_RESULTS = res

    s = np.zeros((B, KD), np.float32)
    for r in res.results:
        s += r["out"].astype(np.float32)
    s = s.reshape(B, K, D)
    sq = np.sum(np.square(s), axis=-1, keepdims=True) + EPS
    v = (np.sqrt(sq) / (1.0 + sq)) * s
    return v.astype(np.float32)


if __name__ == "__main__":
    rng = np.random.default_rng(0)
    x = rng.standard_normal((B, N, I), dtype=np.float32)
    W = (rng.standard_normal((N, K, D, I), dtype=np.float32) * 0.05).astype(np.float32)
    R = rng.standard_normal((N, K), dtype=np.float32)
    out = kernel(x, W, R)
    print("out", out.shape, out.dtype, float(np.abs(out).mean()))
